# revision 14
# baseline (speedup 1.0000x reference)
"""BinaryLinear kernel for 8 Trainium2 NeuronCores.

y = x @ (scale * sign(weight))^T,  x:[8192,4096] f32, weight:[4096,4096] f32.

Strategy: data-parallel token split (1024 tokens/core), weight replicated.
Mixed-precision contraction, all quantization done on HOST:
  - First K16=2048 contraction rows in fp16 (exact sign weights, fp16 x).
  - Remaining K8=2048 rows in fp8 e4m3 with DoubleRow perf mode (2 k-subtiles
    per matmul instruction; measured 216 ns per 256-deep matmul = 1.93x the
    fp16 MAC rate).
Measured (numpy, exact model of the HW math, and confirmed on HW to ~1e-5):
rel err 1.955e-2 vs the fp32 reference -- under the 2e-2 gate.  The device
program is pure DMA + matmul + scaled PSUM drain (no on-device casts).

Per core, per output slab of 512 outs: 8 token-tiles accumulate in lockstep
across the 8 PSUM banks (k-outer), fp16 chunks first then fp8 DoubleRow
chunks, so each weight chunk is consumed by 8 matmuls right after its DMA
lands.  Stores ride the gpsimd SWDGE ring so they never block weight
prefetch on the sync ring (except the last slab, which uses the idle sync
ring).
"""

import numpy as np

TOKENS = 8192
IN_F = 4096
OUT_F = 4096
N_CORES = 8
TS = TOKENS // N_CORES  # tokens per core

P = 128        # partitions
N_TILE = 512   # matmul moving free dim (one PSUM bank of f32)
K16 = 2048     # fp16 contraction rows
K8 = IN_F - K16  # 1792 fp8 rows
C16 = K16 // P          # 18 fp16 k-chunks
C8 = K8 // (2 * P)      # 7 fp8 double-row k-chunks (256 rows each)
T_TILES = TS // P            # 8
O_TILES = OUT_F // N_TILE    # 8
PSUM_BUFS = 8


def _build_program(scale: float):
    import concourse.bacc as bacc
    import concourse.mybir as mybir
    import concourse.tile as tile

    fp32 = mybir.dt.float32
    fp16 = mybir.dt.float16
    fp8 = mybir.dt.float8e4
    DR = mybir.MatmulPerfMode.DoubleRow

    nc = bacc.Bacc(
        "TRN2",
        target_bir_lowering=False,
        debug=False,
        num_devices=N_CORES,
    )
    x16_d = nc.dram_tensor("x16", [K16, TS], fp16, kind="ExternalInput").ap()
    x8_d = nc.dram_tensor("x8", [C8 * P, 2, TS], fp8, kind="ExternalInput").ap()
    w16_d = nc.dram_tensor(
        "w16", [O_TILES * C16 * P, N_TILE], fp16, kind="ExternalInput"
    ).ap()
    w8_d = nc.dram_tensor(
        "w8", [O_TILES * C8 * P, 2, N_TILE], fp8, kind="ExternalInput"
    ).ap()
    y_d = nc.dram_tensor("y", [TS, OUT_F], fp32, kind="ExternalOutput").ap()

    scratch_d = nc.dram_tensor("scratch", [P, N_TILE], fp32, kind="Internal").ap()

    with tile.TileContext(nc) as tc:
        with (
            tc.tile_pool(name="x16res", bufs=C16) as x16_pool,
            tc.tile_pool(name="x8res", bufs=C8) as x8_pool,
            tc.tile_pool(name="w16stage", bufs=2 * C16 + 4) as w16_pool,
            tc.tile_pool(name="w8stage", bufs=2 * C8 + 4) as w8_pool,
            tc.tile_pool(name="ostage", bufs=8) as ostage_pool,
            tc.tile_pool(name="warm", bufs=1) as warm_pool,
            tc.tile_pool(name="psum", bufs=PSUM_BUFS, space="PSUM") as psum_pool,
        ):
            # Short warm-up: ~8 dummy matmuls overlap the fixed DMA/engine
            # startup window (~10us) so the PE HAM clock-gate reaches
            # 2.4 GHz right as the first real matmul's data lands.  Chain
            # ends in a store to an internal scratch tensor so nothing here
            # is dead code.
            warm_h = warm_pool.tile([P, N_TILE], fp16)
            nc.vector.memset(warm_h[:], 0.0)
            warm_ps = psum_pool.tile([P, N_TILE], fp32, tag="ps", name="warm_ps")
            N_WARM = 8
            for i in range(N_WARM):
                nc.tensor.matmul(
                    warm_ps[:],
                    warm_h[:, 0:P],
                    warm_h[:],
                    start=(i == 0),
                    stop=(i == N_WARM - 1),
                )
            warm_o = warm_pool.tile([P, N_TILE], fp32)
            nc.vector.tensor_copy(warm_o[:], warm_ps[:])
            nc.gpsimd.dma_start(scratch_d[:], warm_o[:])

            def load_w16(o, c):
                wt = w16_pool.tile([P, N_TILE], fp16, tag="w16c", name="w16c")
                r = (o * C16 + c) * P
                nc.sync.dma_start(wt[:], w16_d[r : r + P, :])
                return wt

            def load_w8(o, c):
                wt = w8_pool.tile([P, 2, N_TILE], fp8, tag="w8c", name="w8c")
                r = (o * C8 + c) * P
                nc.sync.dma_start(wt[:], w8_d[r : r + P, :, :])
                return wt

            # Phase A: interleave resident x loads with the first slab's
            # weight chunks so the PE starts as soon as pair 0 lands.  The
            # first x16 chunk is split so the first matmul only waits on a
            # 32 KB transfer.  fp8 chunks interleave with the fp16 stream
            # (they are only needed once the slab's fp16 matmuls finish).
            xs16 = []
            xs8 = []
            wb16 = []
            wb8 = []
            for c in range(C16):
                x16t = x16_pool.tile([P, TS], fp16, tag="x16t", name="x16t")
                if c == 0:
                    wb16.append(load_w16(0, 0))
                    nc.sync.dma_start(x16t[:, 0:P], x16_d[0:P, 0:P])
                    nc.sync.dma_start(x16t[:, P:TS], x16_d[0:P, P:TS])
                else:
                    nc.sync.dma_start(x16t[:], x16_d[c * P : (c + 1) * P, :])
                    wb16.append(load_w16(0, c))
                xs16.append(x16t)
            for c in range(C8):
                x8t = x8_pool.tile([P, 2, TS], fp8, tag="x8t", name="x8t")
                nc.sync.dma_start(x8t[:], x8_d[c * P : (c + 1) * P, :, :])
                wb8.append(load_w8(0, c))
                xs8.append(x8t)

            def drain(ps_tile, o, t):
                ot = ostage_pool.tile([P, N_TILE], fp32, tag="ot", name="ot")
                # Stores go on the gpsimd SWDGE ring so they never block
                # weight prefetch on the sync ring -- except the last slab,
                # whose stores use the (by then idle) sync ring so the slow
                # SWDGE drain starts early and leaves the critical path. The
                # very last tile drains in halves so the first half's HBM
                # write receipt overlaps the second half's copy+transfer.
                last = o == O_TILES - 1
                pieces = 4 if (last and t == T_TILES - 1) else 1
                w = N_TILE // pieces
                for p_i in range(pieces):
                    sl = slice(p_i * w, (p_i + 1) * w)
                    if not last:
                        eng = nc.gpsimd
                    else:
                        # alternate rings so the final transfers complete in
                        # parallel
                        eng = nc.sync if p_i % 2 == 0 else nc.gpsimd
                    nc.vector.tensor_scalar_mul(ot[:, sl], ps_tile[:, sl], scale)
                    eng.dma_start(
                        y_d[
                            t * P : (t + 1) * P,
                            o * N_TILE + p_i * w : o * N_TILE + (p_i + 1) * w,
                        ],
                        ot[:, sl],
                    )

            for o in range(O_TILES):
                # Prefetch order: w8 before w16.  The slab's w16 chunks were
                # already prefetched a full slab (~40us) ahead, while the w8
                # chunks were observed arriving just-in-time -- issuing them
                # first removes the DR-section stalls.
                w8c = wb8 if o == 0 else [load_w8(o, c) for c in range(C8)]
                w16c = wb16 if o == 0 else [load_w16(o, c) for c in range(C16)]
                if o < O_TILES - 1:
                    # k-outer with all 8 t-tiles accumulating in lockstep
                    # across the 8 PSUM banks (consumes chunks as they
                    # arrive during the load window).
                    ps = [
                        psum_pool.tile([P, N_TILE], fp32, tag="ps", name="ps")
                        for _ in range(T_TILES)
                    ]
                    for c in range(C16):
                        for t in range(T_TILES):
                            nc.tensor.matmul(
                                ps[t][:],
                                xs16[c][:, t * P : (t + 1) * P],
                                w16c[c][:],
                                start=(c == 0),
                                stop=False,
                            )
                    for c in range(C8):
                        for t in range(T_TILES):
                            nc.tensor.matmul(
                                ps[t][:],
                                xs8[c][:, :, t * P : (t + 1) * P],
                                w8c[c][:],
                                start=False,
                                stop=(c == C8 - 1),
                                perf_mode=DR,
                            )
                    for t in range(T_TILES):
                        drain(ps[t], o, t)
                else:
                    # Last slab runs t-outer so the final drains stagger
                    # instead of all landing after the last matmul.
                    for t in range(T_TILES):
                        pst = psum_pool.tile([P, N_TILE], fp32, tag="ps", name="ps")
                        for c in range(C16):
                            nc.tensor.matmul(
                                pst[:],
                                xs16[c][:, t * P : (t + 1) * P],
                                w16c[c][:],
                                start=(c == 0),
                                stop=False,
                            )
                        for c in range(C8):
                            nc.tensor.matmul(
                                pst[:],
                                xs8[c][:, :, t * P : (t + 1) * P],
                                w8c[c][:],
                                start=False,
                                stop=(c == C8 - 1),
                                perf_mode=DR,
                            )
                        drain(pst, o, t)

    nc.compile()
    return nc


def _prep_weights(weight):
    import ml_dtypes

    wT = np.ascontiguousarray(
        np.where(weight >= 0, np.float32(1.0), np.float32(-1.0)).T
    )  # [IN_F, OUT_F]
    w16 = (
        wT[:K16]
        .astype(np.float16)
        .reshape(C16, P, O_TILES, N_TILE)
        .transpose(2, 0, 1, 3)
        .reshape(O_TILES * C16 * P, N_TILE)
    )
    w8 = (
        wT[K16:]
        .astype(ml_dtypes.float8_e4m3)
        .reshape(C8, 2, P, O_TILES, N_TILE)
        .transpose(3, 0, 2, 1, 4)
        .reshape(O_TILES * C8 * P, 2, N_TILE)
    )
    return np.ascontiguousarray(w16), np.ascontiguousarray(w8)


def _prep_x(xt):
    """xt: [IN_F, TS] f32 (one core's x, transposed)."""
    import ml_dtypes

    x16 = np.ascontiguousarray(xt[:K16].astype(np.float16))
    x8 = np.ascontiguousarray(
        xt[K16:]
        .astype(ml_dtypes.float8_e4m3)
        .reshape(C8, 2, P, TS)
        .transpose(0, 2, 1, 3)
        .reshape(C8 * P, 2, TS)
    )
    return x16, x8


def run(x, weight, scale, trace=False, tmpdir=None):
    from concourse.bass_utils import run_bass_kernel_spmd

    x = np.ascontiguousarray(np.asarray(x, dtype=np.float32))
    weight = np.asarray(weight, dtype=np.float32)
    s = float(np.asarray(scale))

    assert x.shape == (TOKENS, IN_F), x.shape
    assert weight.shape == (OUT_F, IN_F), weight.shape

    nc = _build_program(s)

    w16, w8 = _prep_weights(weight)
    in_maps = []
    for c in range(N_CORES):
        xt = np.ascontiguousarray(x[c * TS : (c + 1) * TS].T)  # [IN_F, TS]
        x16, x8 = _prep_x(xt)
        in_maps.append({"x16": x16, "x8": x8, "w16": w16, "w8": w8})

    res = run_bass_kernel_spmd(
        nc,
        in_maps,
        core_ids=list(range(N_CORES)),
        trace=trace,
        tmpdir=tmpdir,
    )
    y = np.concatenate([res.results[c]["y"] for c in range(N_CORES)], axis=0)
    return y.astype(np.float32, copy=False), res


def kernel(x, weight, scale):
    y, _ = run(x, weight, scale, trace=False)
    return y


# revision 15
# speedup vs baseline: 1.0144x; 1.0144x over previous
"""BinaryLinear kernel for 8 Trainium2 NeuronCores.

y = x @ (scale * sign(weight))^T,  x:[8192,4096] f32, weight:[4096,4096] f32.

Strategy: data-parallel token split (1024 tokens/core), weight replicated.
Mixed-precision contraction, all quantization done on HOST:
  - First K16=2048 contraction rows in fp16 (exact sign weights, fp16 x).
  - Remaining K8=2048 rows in fp8 e4m3 with DoubleRow perf mode (2 k-subtiles
    per matmul instruction; measured 216 ns per 256-deep matmul = 1.93x the
    fp16 MAC rate).
Measured (numpy, exact model of the HW math, and confirmed on HW to ~1e-5):
rel err 1.955e-2 vs the fp32 reference -- under the 2e-2 gate.  The device
program is pure DMA + matmul + scaled PSUM drain (no on-device casts).

Per core, per output slab of 512 outs: 8 token-tiles accumulate in lockstep
across the 8 PSUM banks (k-outer), fp16 chunks first then fp8 DoubleRow
chunks, so each weight chunk is consumed by 8 matmuls right after its DMA
lands.  Stores ride the gpsimd SWDGE ring so they never block weight
prefetch on the sync ring (except the last slab, which uses the idle sync
ring).
"""

import numpy as np

TOKENS = 8192
IN_F = 4096
OUT_F = 4096
N_CORES = 8
TS = TOKENS // N_CORES  # tokens per core

P = 128        # partitions
N_TILE = 512   # matmul moving free dim (one PSUM bank of f32)
K16 = 2048     # fp16 contraction rows
K8 = IN_F - K16  # 1792 fp8 rows
C16 = K16 // P          # 18 fp16 k-chunks
C8 = K8 // (2 * P)      # 7 fp8 double-row k-chunks (256 rows each)
T_TILES = TS // P            # 8
O_TILES = OUT_F // N_TILE    # 8
PSUM_BUFS = 8


def _build_program(scale: float):
    import concourse.bacc as bacc
    import concourse.mybir as mybir
    import concourse.tile as tile

    fp32 = mybir.dt.float32
    fp16 = mybir.dt.float16
    fp8 = mybir.dt.float8e4
    DR = mybir.MatmulPerfMode.DoubleRow

    nc = bacc.Bacc(
        "TRN2",
        target_bir_lowering=False,
        debug=False,
        num_devices=N_CORES,
    )
    x16_d = nc.dram_tensor("x16", [K16, TS], fp16, kind="ExternalInput").ap()
    x8_d = nc.dram_tensor("x8", [C8 * P, 2, TS], fp8, kind="ExternalInput").ap()
    w16_d = nc.dram_tensor(
        "w16", [O_TILES * C16 * P, N_TILE], fp16, kind="ExternalInput"
    ).ap()
    w8_d = nc.dram_tensor(
        "w8", [O_TILES * C8 * P, 2, N_TILE], fp8, kind="ExternalInput"
    ).ap()
    y_d = nc.dram_tensor("y", [TS, OUT_F], fp32, kind="ExternalOutput").ap()

    scratch_d = nc.dram_tensor("scratch", [P, N_TILE], fp32, kind="Internal").ap()

    with tile.TileContext(nc) as tc:
        with (
            tc.tile_pool(name="x16res", bufs=C16) as x16_pool,
            tc.tile_pool(name="x8res", bufs=C8) as x8_pool,
            tc.tile_pool(name="w16stage", bufs=2 * C16 + 4) as w16_pool,
            tc.tile_pool(name="w8stage", bufs=2 * C8 + 4) as w8_pool,
            tc.tile_pool(name="ostage", bufs=8) as ostage_pool,
            tc.tile_pool(name="warm", bufs=1) as warm_pool,
            tc.tile_pool(name="psum", bufs=PSUM_BUFS, space="PSUM") as psum_pool,
        ):
            # Short warm-up: ~8 dummy matmuls overlap the fixed DMA/engine
            # startup window (~10us) so the PE HAM clock-gate reaches
            # 2.4 GHz right as the first real matmul's data lands.  Chain
            # ends in a store to an internal scratch tensor so nothing here
            # is dead code.
            warm_h = warm_pool.tile([P, N_TILE], fp16)
            nc.vector.memset(warm_h[:], 0.0)
            warm_ps = psum_pool.tile([P, N_TILE], fp32, tag="ps", name="warm_ps")
            N_WARM = 8
            for i in range(N_WARM):
                nc.tensor.matmul(
                    warm_ps[:],
                    warm_h[:, 0:P],
                    warm_h[:],
                    start=(i == 0),
                    stop=(i == N_WARM - 1),
                )
            warm_o = warm_pool.tile([P, N_TILE], fp32)
            nc.vector.tensor_copy(warm_o[:], warm_ps[:])
            nc.gpsimd.dma_start(scratch_d[:], warm_o[:])

            def load_w16(o, c):
                wt = w16_pool.tile([P, N_TILE], fp16, tag="w16c", name="w16c")
                r = (o * C16 + c) * P
                nc.sync.dma_start(wt[:], w16_d[r : r + P, :])
                return wt

            def load_w8(o, c):
                wt = w8_pool.tile([P, 2, N_TILE], fp8, tag="w8c", name="w8c")
                r = (o * C8 + c) * P
                nc.sync.dma_start(wt[:], w8_d[r : r + P, :, :])
                return wt

            # Phase A: interleave resident x loads with the first slab's
            # weight chunks so the PE starts as soon as pair 0 lands.  The
            # first x16 chunk is split so the first matmul only waits on a
            # 32 KB transfer.  fp8 chunks interleave with the fp16 stream
            # (they are only needed once the slab's fp16 matmuls finish).
            xs16 = []
            xs8 = []
            wb16 = []
            wb8 = []
            for c in range(C16):
                x16t = x16_pool.tile([P, TS], fp16, tag="x16t", name="x16t")
                if c == 0:
                    wb16.append(load_w16(0, 0))
                    nc.sync.dma_start(x16t[:, 0:P], x16_d[0:P, 0:P])
                    nc.sync.dma_start(x16t[:, P:TS], x16_d[0:P, P:TS])
                else:
                    nc.sync.dma_start(x16t[:], x16_d[c * P : (c + 1) * P, :])
                    wb16.append(load_w16(0, c))
                xs16.append(x16t)
            for c in range(C8):
                x8t = x8_pool.tile([P, 2, TS], fp8, tag="x8t", name="x8t")
                nc.sync.dma_start(x8t[:], x8_d[c * P : (c + 1) * P, :, :])
                wb8.append(load_w8(0, c))
                xs8.append(x8t)

            def drain(ps_tile, o, t):
                ot = ostage_pool.tile([P, N_TILE], fp32, tag="ot", name="ot")
                # Stores go on the gpsimd SWDGE ring so they never block
                # weight prefetch on the sync ring -- except the last slab,
                # whose stores use the (by then idle) sync ring so the slow
                # SWDGE drain starts early and leaves the critical path. The
                # very last tile drains in halves so the first half's HBM
                # write receipt overlaps the second half's copy+transfer.
                last = o == O_TILES - 1
                eng = nc.sync if last else nc.gpsimd
                pieces = 2 if (last and t == T_TILES - 1) else 1
                w = N_TILE // pieces
                for p_i in range(pieces):
                    sl = slice(p_i * w, (p_i + 1) * w)
                    nc.vector.tensor_scalar_mul(ot[:, sl], ps_tile[:, sl], scale)
                    eng.dma_start(
                        y_d[
                            t * P : (t + 1) * P,
                            o * N_TILE + p_i * w : o * N_TILE + (p_i + 1) * w,
                        ],
                        ot[:, sl],
                    )

            for o in range(O_TILES):
                # Prefetch order: w8 before w16.  The slab's w16 chunks were
                # already prefetched a full slab (~40us) ahead, while the w8
                # chunks were observed arriving just-in-time -- issuing them
                # first removes the DR-section stalls.
                w8c = wb8 if o == 0 else [load_w8(o, c) for c in range(C8)]
                w16c = wb16 if o == 0 else [load_w16(o, c) for c in range(C16)]
                if o < O_TILES - 1:
                    # k-outer with all 8 t-tiles accumulating in lockstep
                    # across the 8 PSUM banks (consumes chunks as they
                    # arrive during the load window).
                    ps = [
                        psum_pool.tile([P, N_TILE], fp32, tag="ps", name="ps")
                        for _ in range(T_TILES)
                    ]
                    for c in range(C16):
                        for t in range(T_TILES):
                            nc.tensor.matmul(
                                ps[t][:],
                                xs16[c][:, t * P : (t + 1) * P],
                                w16c[c][:],
                                start=(c == 0),
                                stop=False,
                            )
                    for c in range(C8):
                        for t in range(T_TILES):
                            nc.tensor.matmul(
                                ps[t][:],
                                xs8[c][:, :, t * P : (t + 1) * P],
                                w8c[c][:],
                                start=False,
                                stop=(c == C8 - 1),
                                perf_mode=DR,
                            )
                    for t in range(T_TILES):
                        drain(ps[t], o, t)
                else:
                    # Last slab runs t-outer so the final drains stagger
                    # instead of all landing after the last matmul.
                    for t in range(T_TILES):
                        pst = psum_pool.tile([P, N_TILE], fp32, tag="ps", name="ps")
                        for c in range(C16):
                            nc.tensor.matmul(
                                pst[:],
                                xs16[c][:, t * P : (t + 1) * P],
                                w16c[c][:],
                                start=(c == 0),
                                stop=False,
                            )
                        for c in range(C8):
                            nc.tensor.matmul(
                                pst[:],
                                xs8[c][:, :, t * P : (t + 1) * P],
                                w8c[c][:],
                                start=False,
                                stop=(c == C8 - 1),
                                perf_mode=DR,
                            )
                        drain(pst, o, t)

    nc.compile()
    return nc


def _prep_weights(weight):
    import ml_dtypes

    wT = np.ascontiguousarray(
        np.where(weight >= 0, np.float32(1.0), np.float32(-1.0)).T
    )  # [IN_F, OUT_F]
    w16 = (
        wT[:K16]
        .astype(np.float16)
        .reshape(C16, P, O_TILES, N_TILE)
        .transpose(2, 0, 1, 3)
        .reshape(O_TILES * C16 * P, N_TILE)
    )
    w8 = (
        wT[K16:]
        .astype(ml_dtypes.float8_e4m3)
        .reshape(C8, 2, P, O_TILES, N_TILE)
        .transpose(3, 0, 2, 1, 4)
        .reshape(O_TILES * C8 * P, 2, N_TILE)
    )
    return np.ascontiguousarray(w16), np.ascontiguousarray(w8)


def _prep_x(xt):
    """xt: [IN_F, TS] f32 (one core's x, transposed)."""
    import ml_dtypes

    x16 = np.ascontiguousarray(xt[:K16].astype(np.float16))
    x8 = np.ascontiguousarray(
        xt[K16:]
        .astype(ml_dtypes.float8_e4m3)
        .reshape(C8, 2, P, TS)
        .transpose(0, 2, 1, 3)
        .reshape(C8 * P, 2, TS)
    )
    return x16, x8


def run(x, weight, scale, trace=False, tmpdir=None):
    from concourse.bass_utils import run_bass_kernel_spmd

    x = np.ascontiguousarray(np.asarray(x, dtype=np.float32))
    weight = np.asarray(weight, dtype=np.float32)
    s = float(np.asarray(scale))

    assert x.shape == (TOKENS, IN_F), x.shape
    assert weight.shape == (OUT_F, IN_F), weight.shape

    nc = _build_program(s)

    w16, w8 = _prep_weights(weight)
    in_maps = []
    for c in range(N_CORES):
        xt = np.ascontiguousarray(x[c * TS : (c + 1) * TS].T)  # [IN_F, TS]
        x16, x8 = _prep_x(xt)
        in_maps.append({"x16": x16, "x8": x8, "w16": w16, "w8": w8})

    res = run_bass_kernel_spmd(
        nc,
        in_maps,
        core_ids=list(range(N_CORES)),
        trace=trace,
        tmpdir=tmpdir,
    )
    y = np.concatenate([res.results[c]["y"] for c in range(N_CORES)], axis=0)
    return y.astype(np.float32, copy=False), res


def kernel(x, weight, scale):
    y, _ = run(x, weight, scale, trace=False)
    return y
